# revision 1
# baseline (speedup 1.0000x reference)
"""Trainium2 Bass kernel for ClassForgeEnsembleGNN (SAGE -> GAT -> RGCN ensemble).

Strategy (8 NeuronCores, SPMD):
  - Nodes partitioned into 8 contiguous shards (6250 each); each core owns the
    edges whose target is in its shard.  Weights replicated; x1/x2 node
    features all-gathered between stages (device collectives).
  - Per-edge source rows fetched with batched GPSIMD dma_gather (int16
    indices; tables addressed through two views split at 32768; each
    (block, half) edge group padded to a uniform tile count across cores).
  - Scatter-aggregation via selection-matrix matmuls.  S built on DVE with
    all-bf16 operands (4x DVE mode); per-edge normalizers (1/cnt for SAGE,
    relation-mask/cnt_r for RGCN) folded into the S build scalar on the host.
  - SAGE/RGCN aggregate feature-major (lhsT = gathered rows) so no
    per-block transposes are needed; RGCN accumulates root + all relation
    terms in a single PSUM chain.
  - GAT: per-edge a_d via a small matmul against host-precomputed
    transposed one-hot tiles streamed from DRAM; leaky-relu/exp on ScalarE;
    exp(logit) folded into the S build; self-loops as diagonal S tiles.
  - One-block deferred emission of the accumulation matmuls/epilogues keeps
    the PE queue from stalling on cross-engine round trips.
"""

import sys
import os

for _p in ("/opt/trn_rl_repo", "/root/.axon_site/_ro/trn_rl_repo"):
    if os.path.isdir(_p) and _p not in sys.path:
        sys.path.append(_p)

import numpy as np
import ml_dtypes

import concourse.bacc as bacc
import concourse.bass as bass
import concourse.mybir as mybir
import concourse.tile as tile
from concourse.bass_utils import run_bass_kernel_spmd
from concourse.masks import make_identity

P = 128
NCORES = 8
N = 50000
E = 400000
D = 128
H = 2
R = 5
NEG = 0.2
SH = N // NCORES            # 6250 nodes per shard
B = (SH + P - 1) // P       # 49 blocks (last has 106 valid nodes)
HSPLIT = 32768              # int16-safe table split
G_E = 8                     # tiles per dma_gather group, 128-col bf16 rows
G_G = 8                     # tiles per group, 256-col bf16 rows
XSTRIDE = 132               # x1 row block stride: [x1(128) | 1 | as0 | as1 | pad]

f32 = mybir.dt.float32
bf16 = mybir.dt.bfloat16
i32 = mybir.dt.int32
i16 = mybir.dt.int16
AF = mybir.ActivationFunctionType
ALU = mybir.AluOpType
BF = ml_dtypes.bfloat16


def _pack_stage(src_pc, dst_pc, rel_pc, nrel, wslot_fn):
    """Lay out each core's edges into uniform tile slots.

    Grouping: (half = src >= HSPLIT, block = dst_local // 128 [, relation]).
    Sections (half, block) are tile-aligned; relation runs inside a section
    are slot-aligned (uniform across cores).

    wslot_fn(core, dst_local, rel) -> per-slot scalar folded into the S
    build (SAGE: 1/cnt[dst]; RGCN: 1/cnt_r[dst]).

    Returns dict with per-stream tile counts/offsets, run metadata, and
    per-core packed arrays.
    """
    counts = np.zeros((NCORES, 2, B, nrel), np.int64)
    for k in range(NCORES):
        h = (src_pc[k] >= HSPLIT).astype(np.int64)
        blk = dst_pc[k] // P
        r = rel_pc[k] if nrel > 1 else np.zeros(len(src_pc[k]), np.int64)
        np.add.at(counts[k], (h, blk, r), 1)
    caps = counts.max(0)                       # [2, B, nrel] slot caps
    run_off = np.zeros((2, B, nrel + 1), np.int64)
    np.cumsum(caps, axis=2, out=run_off[:, :, 1:])
    sec_slots = run_off[:, :, nrel]            # [2, B]
    sec_tiles = (sec_slots + P - 1) // P       # [2, B]
    tile_off = np.zeros((2, B + 1), np.int64)  # tile offset within stream
    np.cumsum(sec_tiles, axis=1, out=tile_off[:, 1:])
    T = tile_off[:, B]                         # [2] tiles per stream

    # per-core packed arrays
    cores = []
    for k in range(NCORES):
        h = (src_pc[k] >= HSPLIT).astype(np.int64)
        blk = dst_pc[k] // P
        r = rel_pc[k] if nrel > 1 else np.zeros(len(src_pc[k]), np.int64)
        gid = (h * B + blk) * nrel + r
        order = np.argsort(gid, kind="stable")
        gs = gid[order]
        cnt_flat = counts[k].reshape(-1)
        starts = np.concatenate([[0], np.cumsum(cnt_flat)])[:-1]
        rank = np.arange(len(gs)) - starts[gs]
        hh, rest = gs // (B * nrel), gs % (B * nrel)
        bb, rr = rest // nrel, rest % nrel
        slot = tile_off[hh, bb] * P + run_off[hh, bb, rr] + rank
        w_all = wslot_fn(k, dst_pc[k], rel_pc[k] if nrel > 1 else None)
        packed = []
        for hv in range(2):
            n_slots = int(T[hv]) * P
            idx = np.zeros(n_slots, np.int64)
            dl = np.full(n_slots, -1.0, np.float32)
            wv = np.zeros(n_slots, np.float32)
            sel = hh == hv
            s = slot[sel]  # slot is relative to its stream's start
            idx[s] = src_pc[k][order][sel] - (HSPLIT if hv else 0)
            dl[s] = (dst_pc[k][order][sel] % P).astype(np.float32)
            wv[s] = w_all[order][sel]
            # idx packed for dma_gather: flat i -> (channel i%16, col i//16),
            # replicated down the 8 Q7 core groups
            m = idx.reshape(-1, 16).T.astype(np.int16)
            idx16 = np.tile(m, (8, 1))
            dlp = np.ascontiguousarray(dl.reshape(int(T[hv]), P).T)
            if nrel > 1:
                # per-(slot, relation) masked weight for the S build scalar2
                rw = np.zeros((n_slots, nrel), np.float32)
                rw[s, rr[sel]] = w_all[order][sel]
                rwp = np.ascontiguousarray(
                    rw.reshape(int(T[hv]), P, nrel).transpose(1, 0, 2)
                    .reshape(P, int(T[hv]) * nrel))
                stp = None
                ewp = None
            else:
                rwp = None
                ewp = np.ascontiguousarray(wv.reshape(int(T[hv]), P).T)
                # transposed one-hot tiles ST[n, t*128+e] = (dl[t,e] == n)
                dlt = dl.reshape(int(T[hv]), P)
                st = (dlt[None, :, :] == np.arange(P, dtype=np.float32)
                      [:, None, None])
                stp = np.ascontiguousarray(
                    st.reshape(P, int(T[hv]) * P)
                    if st.size else np.zeros((P, 0)))
                stp = stp.astype(BF)
            packed.append((np.ascontiguousarray(idx16), dlp, rwp, ewp, stp))
        cores.append(packed)

    return dict(caps=caps, run_off=run_off, sec_tiles=sec_tiles,
                tile_off=tile_off, T=(int(T[0]), int(T[1])), cores=cores)


def _preprocess(x, edge_index, edge_type):
    src = edge_index[0].astype(np.int64)
    dst = edge_index[1].astype(np.int64)
    et = edge_type.astype(np.int64)

    cnt = np.bincount(dst, minlength=N).astype(np.float32)
    wrec_all = 1.0 / np.maximum(cnt, 1.0)
    cnt_r = np.zeros((R, N), np.float32)
    for r in range(R):
        cnt_r[r] = np.bincount(dst[et == r], minlength=N)
    wrrec_all = 1.0 / np.maximum(cnt_r, 1.0)

    shard_of = dst // SH
    s_src, s_dst, r_rel = [], [], []
    for k in range(NCORES):
        sel = shard_of == k
        s_src.append(src[sel])
        s_dst.append(dst[sel] - k * SH)
        r_rel.append(et[sel])

    def w_sage(k, dst_local, rel):
        return wrec_all[k * SH + dst_local]

    def w_rgcn(k, dst_local, rel):
        return wrrec_all[rel, k * SH + dst_local]

    edges = _pack_stage(s_src, s_dst, [None] * NCORES, 1, w_sage)
    rgcn = _pack_stage(s_src, s_dst, r_rel, R, w_rgcn)
    return dict(edges=edges, rgcn=rgcn)


def _rgcn_runs(rgcn):
    """Per block: list of (h, stream_tile, r) matmul jobs."""
    runs_per_block = []
    for b in range(B):
        runs = []
        for h in range(2):
            t0 = int(rgcn["tile_off"][h, b])
            ntile = int(rgcn["sec_tiles"][h, b])
            for tl in range(ntile):
                s0, s1 = tl * P, (tl + 1) * P
                for r in range(R):
                    lo = int(rgcn["run_off"][h, b, r])
                    hi = int(rgcn["run_off"][h, b, r + 1])
                    if lo < s1 and hi > s0:
                        runs.append((h, t0 + tl, r))
        runs_per_block.append(runs)
    return runs_per_block


def _build_program(pp):
    edges, rgcn = pp["edges"], pp["rgcn"]
    rgcn_runs = _rgcn_runs(rgcn)

    nc = bacc.Bacc("TRN2", target_bir_lowering=False, debug=False,
                   num_devices=NCORES, num_swdge_queues=4)

    xb_dram = nc.dram_tensor("xb", [N, D], bf16, kind="ExternalInput")
    xt_dram = nc.dram_tensor("xt", [B, P, P], bf16, kind="ExternalInput")
    meta = {}
    for h in range(2):
        T = edges["T"][h]
        if T:
            meta[f"ei{h}"] = nc.dram_tensor(
                f"ei{h}", [P, T * 8], i16, kind="ExternalInput")
            meta[f"ed{h}"] = nc.dram_tensor(
                f"ed{h}", [P, T], f32, kind="ExternalInput")
            meta[f"ew{h}"] = nc.dram_tensor(
                f"ew{h}", [P, T], f32, kind="ExternalInput")
            meta[f"st{h}"] = nc.dram_tensor(
                f"st{h}", [P, T * P], bf16, kind="ExternalInput")
        T = rgcn["T"][h]
        if T:
            meta[f"ri{h}"] = nc.dram_tensor(
                f"ri{h}", [P, T * 8], i16, kind="ExternalInput")
            meta[f"rd{h}"] = nc.dram_tensor(
                f"rd{h}", [P, T], f32, kind="ExternalInput")
            meta[f"rw{h}"] = nc.dram_tensor(
                f"rw{h}", [P, T * R], f32, kind="ExternalInput")
    wsl = nc.dram_tensor("wsl", [D, D], bf16, kind="ExternalInput")
    wsr = nc.dram_tensor("wsr", [D, D], bf16, kind="ExternalInput")
    bs = nc.dram_tensor("bs", [P, 1], f32, kind="ExternalInput")
    vsd = nc.dram_tensor("vsd", [D, 4], f32, kind="ExternalInput")
    wg0 = nc.dram_tensor("wg0", [D, D], bf16, kind="ExternalInput")
    wg1 = nc.dram_tensor("wg1", [D, D], bf16, kind="ExternalInput")
    bg = nc.dram_tensor("bg", [P, 1], f32, kind="ExternalInput")
    wroot = nc.dram_tensor("wroot", [D, D], bf16, kind="ExternalInput")
    wr_d = nc.dram_tensor("wr", [R, D, D], bf16, kind="ExternalInput")
    br = nc.dram_tensor("br", [P, 1], f32, kind="ExternalInput")
    out_dram = nc.dram_tensor("out", [SH, 3 * D], f32, kind="ExternalOutput")

    rg = [list(range(NCORES))]
    qrr = [0]

    def next_q():
        q = qrr[0]
        qrr[0] = (q + 1) % 4
        return q

    with tile.TileContext(nc) as tc:
        with (
            tc.tile_pool(name="const", bufs=1) as cb,
            tc.tile_pool(name="sbuf", bufs=3) as sb,
            tc.tile_pool(name="psum", bufs=1, space="PSUM") as ps,
            tc.tile_pool(name="dram", bufs=1, space="DRAM") as dr,
        ):
            identb = cb.tile([P, P], bf16)
            make_identity(nc, identb[:])
            ident = cb.tile([P, P], f32)
            make_identity(nc, ident[:])
            iota_i = cb.tile([P, P], i32)
            nc.gpsimd.iota(iota_i[:], pattern=[[1, P]], base=0,
                           channel_multiplier=0)
            iota_b = cb.tile([P, P], bf16)
            nc.vector.tensor_copy(iota_b[:], iota_i[:])
            iota_ci = cb.tile([P, 1], i32)
            nc.gpsimd.iota(iota_ci[:], pattern=[[1, 1]], base=0,
                           channel_multiplier=1)
            iota_c = cb.tile([P, 1], f32)
            nc.vector.tensor_copy(iota_c[:], iota_ci[:])

            def load_const(name, dram, dtype):
                t = cb.tile(list(dram.shape), dtype, name=name)
                nc.sync.dma_start(t[:], dram[:])
                return t

            wsl_sb = load_const("wsl_sb", wsl, bf16)
            wsr_sb = load_const("wsr_sb", wsr, bf16)
            bs_sb = load_const("bs_sb", bs, f32)
            vsd_sb = load_const("vsd_sb", vsd, f32)
            wg0_sb = load_const("wg0_sb", wg0, bf16)
            wg1_sb = load_const("wg1_sb", wg1, bf16)
            bg_sb = load_const("bg_sb", bg, f32)
            wroot_sb = load_const("wroot_sb", wroot, bf16)
            br_sb = load_const("br_sb", br, f32)
            wr_sb = cb.tile([P, R * D], bf16)
            for r in range(R):
                nc.sync.dma_start(wr_sb[:, r * D:(r + 1) * D], wr_d[r, :, :])
            meta_sb = {}
            for name, dram in meta.items():
                if name.startswith("st"):
                    continue  # streamed, not resident
                t = cb.tile(list(dram.shape), dram.dtype, name=f"{name}_sb")
                nc.sync.dma_start(t[:], dram[:])
                meta_sb[name] = t

            # persistent per-shard state
            x1nb_sb = cb.tile([P, B * XSTRIDE], bf16)  # [x1 | 1 | as] / block
            adb_sb = cb.tile([P, 2 * B], bf16)   # a_d bf16 / block
            exl_sb = cb.tile([P, 2 * B], f32)    # exp(leaky(as+ad)) / block
            x2Tb_sb = cb.tile([P, B * P], bf16)  # x2 transposed bf16

            cc1_in = dr.tile([SH, 256], bf16)
            cc1_out = dr.tile([N, 256], bf16, addr_space="Shared")
            cc2_in = dr.tile([SH, D], bf16)
            cc2_out = dr.tile([N, D], bf16, addr_space="Shared")

            # ---------- gather-group machinery ----------
            def make_stage(key, tag, d_, table_views, width, gtiles, bufs):
                state = {}

                def get_tile(h, t):
                    g = t // gtiles
                    k2 = (h, g)
                    if k2 not in state:
                        T = d_["T"][h]
                        g0 = g * gtiles
                        gn = min(gtiles, T - g0)
                        xg = sb.tile([P, gtiles, width], bf16,
                                     tag=tag, bufs=bufs)
                        ni = gn * P
                        nc.gpsimd.dma_gather(
                            xg[:, 0:gn, :], table_views[h],
                            meta_sb[f"{key}i{h}"][:, g0 * 8:(g0 + gn) * 8],
                            ni, ni, width, queue_num=next_q())
                        state[k2] = xg
                    return state[k2][:, t - g * gtiles, :]

                return get_tile

            # streamed ST one-hot tiles, group-batched DRAM loads
            GS = 8
            st_state = {}

            def st_tile(h, t):
                g = t // GS
                k2 = (h, g)
                if k2 not in st_state:
                    T = edges["T"][h]
                    g0 = g * GS
                    gn = min(GS, T - g0)
                    stg = sb.tile([P, GS, P], bf16, tag="stg", bufs=3)
                    nc.sync.dma_start(
                        stg[:, 0:gn, :],
                        meta[f"st{h}"][:, g0 * P:(g0 + gn) * P])
                    st_state[k2] = stg
                return st_state[k2][:, t - g * GS, :]

            def block_tiles(d_, b):
                return [(h, t) for h in range(2)
                        for t in range(int(d_["tile_off"][h, b]),
                                       int(d_["tile_off"][h, b + 1]))]

            # =================== Stage 1: SAGE ===================
            # per tile: S = onehot(dst) * (1/cnt[dst]) (bf16 4x DVE build),
            # aggT[c, n] += xg^T S (feature-major, no transpose later).
            sc = nc.enter_named_scope("sage", False)
            sage_tile = make_stage(
                "e", "xgs", edges, (xb_dram[0:HSPLIT, :], xb_dram[HSPLIT:N, :]),
                D, G_E, bufs=3)

            def sage_block(b):
                tiles = block_tiles(edges, b)
                pa = ps.tile([P, P], f32, tag="acc", bufs=2)
                for j, (h, t) in enumerate(tiles):
                    xg = sage_tile(h, t)
                    S = sb.tile([P, P], bf16, tag="Se", bufs=6)
                    nc.vector.tensor_scalar(
                        out=S[:], in0=iota_b[:],
                        scalar1=meta_sb[f"ed{h}"][:, t:t + 1],
                        scalar2=meta_sb[f"ew{h}"][:, t:t + 1],
                        op0=ALU.is_equal, op1=ALU.mult)
                    nc.tensor.matmul(pa[:], lhsT=xg, rhs=S[:],
                                     start=(j == 0),
                                     stop=(j == len(tiles) - 1))
                if not tiles:
                    nc.vector.memset(pa[:], 0.0)
                return pa

            def sage_epilogue(b, pa):
                vld = min(P, SH - b * P)
                r0 = b * P
                x0 = b * XSTRIDE
                aggT = sb.tile([P, P], bf16, tag="aggT")
                nc.scalar.copy(aggT[:], pa[:])
                xT = sb.tile([P, P], bf16, tag="xT")
                nc.sync.dma_start(xT[:], xt_dram[b, :, :])
                pm = ps.tile([P, P], f32, tag="mm", bufs=2)
                nc.tensor.matmul(pm[:], lhsT=wsl_sb[:], rhs=aggT[:],
                                 start=True, stop=False)
                nc.tensor.matmul(pm[:], lhsT=wsr_sb[:], rhs=xT[:],
                                 start=False, stop=True)
                x1T = sb.tile([P, P], f32, tag="x1T")
                nc.scalar.activation(x1T[:], pm[:], AF.Relu,
                                     bias=bs_sb[:, 0:1], scale=1.0)
                pasd = ps.tile([P, 4], f32, tag="c", bufs=1)
                nc.tensor.matmul(pasd[:], lhsT=x1T[:], rhs=vsd_sb[:],
                                 start=True, stop=True)
                asd4 = sb.tile([P, 4], f32, tag="asd4")
                nc.scalar.copy(asd4[:], pasd[:])
                nc.scalar.copy(adb_sb[:, 2 * b:2 * b + 2], pasd[:, 2:4])
                evl = sb.tile([P, 2], f32, tag="evl")
                nc.vector.tensor_add(evl[:], asd4[:, 0:2], asd4[:, 2:4])
                ev2 = sb.tile([P, 2], f32, tag="evl2")
                nc.vector.tensor_scalar(out=ev2[:], in0=evl[:], scalar1=NEG,
                                        scalar2=None, op0=ALU.mult)
                lvl = sb.tile([P, 2], f32, tag="lvl")
                nc.vector.tensor_tensor(out=lvl[:], in0=ev2[:], in1=evl[:],
                                        op=ALU.max)
                nc.scalar.activation(exl_sb[:, 2 * b:2 * b + 2], lvl[:],
                                     AF.Exp)
                ptr2 = ps.tile([P, P], f32, tag="mm", bufs=2)
                nc.tensor.transpose(ptr2[:], x1T[:], ident[:])
                nc.scalar.copy(x1nb_sb[:, x0:x0 + P], ptr2[:])
                nc.vector.memset(x1nb_sb[:, x0 + P:x0 + P + 1], 1.0)
                nc.scalar.copy(x1nb_sb[:, x0 + 129:x0 + 131], asd4[:, 0:2])
                x1n = sb.tile([P, P], f32, tag="x1n")
                nc.scalar.copy(x1n[:], ptr2[:])
                nc.sync.dma_start(cc1_in[r0:r0 + vld, 0:131],
                                  x1nb_sb[:vld, x0:x0 + 131])
                nc.sync.dma_start(out_dram[r0:r0 + vld, 0:D], x1n[:vld, :])

            pend = None
            for b in range(B):
                pa = sage_block(b)
                if pend is not None:
                    sage_epilogue(b - 1, pend)
                pend = pa
            sage_epilogue(B - 1, pend)
            nc.leave_named_scope("sage", sc[0], False)

            sc = nc.enter_named_scope("ag1", False)
            nc.gpsimd.collective_compute(
                "AllGather", ALU.bypass, replica_groups=rg,
                ins=[cc1_in[:]], outs=[cc1_out[:]])
            nc.leave_named_scope("ag1", sc[0], False)

            # =================== Stage 2: GAT ===================
            sc = nc.enter_named_scope("gat", False)
            gat_tile = make_stage(
                "e", "xgg", edges, (cc1_out[0:HSPLIT, :], cc1_out[HSPLIT:N, :]),
                256, G_G, bufs=6)

            def gat_block(b):
                """Emit gathers, ST loads, pad_ matmuls, per-edge logits and
                S builds for block b.  Returns deferred matmul program."""
                tiles = block_tiles(edges, b)
                jobs = []
                nt = max(1, len(tiles))
                padp = ps.tile([P, 2 * nt], f32, tag="c", bufs=1)
                for j, (h, t) in enumerate(tiles):
                    stt = st_tile(h, t)
                    nc.tensor.matmul(padp[:, 2 * j:2 * j + 2], lhsT=stt,
                                     rhs=adb_sb[:, 2 * b:2 * b + 2],
                                     start=True, stop=True,
                                     skip_group_check=True)
                pads = sb.tile([P, 2 * nt], f32, tag="pads", bufs=2)
                nc.scalar.copy(pads[:], padp[:])
                for j, (h, t) in enumerate(tiles):
                    xg = gat_tile(h, t)
                    ev = sb.tile([P, 2], f32, tag="ev")
                    nc.vector.tensor_add(ev[:], xg[:, 129:131],
                                         pads[:, 2 * j:2 * j + 2])
                    ev2 = sb.tile([P, 2], f32, tag="ev2")
                    nc.vector.tensor_scalar(out=ev2[:], in0=ev[:], scalar1=NEG,
                                            scalar2=None, op0=ALU.mult)
                    lv = sb.tile([P, 2], f32, tag="lv")
                    nc.vector.tensor_tensor(out=lv[:], in0=ev2[:], in1=ev[:],
                                            op=ALU.max)
                    exf = sb.tile([P, 2], f32, tag="exf", bufs=6)
                    nc.scalar.activation(exf[:], lv[:], AF.Exp)
                    S0 = sb.tile([P, P], bf16, tag="Sg", bufs=56)
                    nc.vector.tensor_scalar(
                        out=S0[:], in0=iota_b[:],
                        scalar1=meta_sb[f"ed{h}"][:, t:t + 1],
                        scalar2=exf[:, 0:1], op0=ALU.is_equal, op1=ALU.mult)
                    S1 = sb.tile([P, P], bf16, tag="Sg", bufs=56)
                    nc.vector.tensor_scalar(
                        out=S1[:], in0=iota_b[:],
                        scalar1=meta_sb[f"ed{h}"][:, t:t + 1],
                        scalar2=exf[:, 1:2], op0=ALU.is_equal, op1=ALU.mult)
                    jobs.append((S0, S1, xg))
                # self-loop tiles: S = diag(exl_h)
                Ss = []
                for hh in range(2):
                    Sd = sb.tile([P, P], bf16, tag="Sg", bufs=56)
                    nc.vector.tensor_scalar(
                        out=Sd[:], in0=iota_b[:], scalar1=iota_c[:, 0:1],
                        scalar2=exl_sb[:, 2 * b + hh:2 * b + hh + 1],
                        op0=ALU.is_equal, op1=ALU.mult)
                    Ss.append(Sd)
                return jobs, Ss

            def gat_flush(b, jobs, Ss):
                vld = min(P, SH - b * P)
                r0 = b * P
                x0 = b * XSTRIDE
                p0 = ps.tile([P, 129], f32, tag="acc", bufs=2)
                p1 = ps.tile([P, 129], f32, tag="acc2", bufs=2)
                n = len(jobs)
                for j, (S0, S1, xg) in enumerate(jobs):
                    nc.tensor.matmul(p0[:], lhsT=S0[:], rhs=xg[:, 0:129],
                                     start=(j == 0), stop=False)
                    nc.tensor.matmul(p1[:], lhsT=S1[:], rhs=xg[:, 0:129],
                                     start=(j == 0), stop=False)
                nc.tensor.matmul(p0[:], lhsT=Ss[0][:],
                                 rhs=x1nb_sb[:, x0:x0 + 129],
                                 start=(n == 0), stop=True)
                nc.tensor.matmul(p1[:], lhsT=Ss[1][:],
                                 rhs=x1nb_sb[:, x0:x0 + 129],
                                 start=(n == 0), stop=True)
                den = sb.tile([P, 2], f32, tag="den")
                nc.scalar.copy(den[:, 0:1], p0[:, 128:129])
                nc.scalar.copy(den[:, 1:2], p1[:, 128:129])
                rec = sb.tile([P, 2], f32, tag="rec")
                nc.vector.reciprocal(rec[:], den[:])
                tb = []
                for hh, pp_ in ((0, p0), (1, p1)):
                    nmh = sb.tile([P, P], bf16, tag=f"nm{hh}")
                    nc.scalar.activation(nmh[:], pp_[:, 0:128], AF.Identity,
                                         scale=rec[:, hh:hh + 1])
                    ptn = ps.tile([P, P], bf16, tag="mm", bufs=2)
                    nc.tensor.transpose(ptn[:], nmh[:], identb[:])
                    tbh = sb.tile([P, P], bf16, tag=f"tb{hh}")
                    nc.scalar.copy(tbh[:], ptn[:])
                    tb.append(tbh)
                px2 = ps.tile([P, P], f32, tag="mm", bufs=2)
                nc.tensor.matmul(px2[:], lhsT=wg0_sb[:], rhs=tb[0][:],
                                 start=True, stop=False)
                nc.tensor.matmul(px2[:], lhsT=wg1_sb[:], rhs=tb[1][:],
                                 start=False, stop=True)
                nc.scalar.activation(x2Tb_sb[:, r0:r0 + P], px2[:], AF.Relu,
                                     bias=bg_sb[:, 0:1], scale=0.5)
                ptx = ps.tile([P, P], bf16, tag="mm", bufs=2)
                nc.tensor.transpose(ptx[:], x2Tb_sb[:, r0:r0 + P], identb[:])
                x2n = sb.tile([P, P], f32, tag="x2n")
                nc.scalar.copy(x2n[:], ptx[:])
                x2nb = sb.tile([P, P], bf16, tag="x2nb")
                nc.scalar.copy(x2nb[:], ptx[:])
                nc.sync.dma_start(cc2_in[r0:r0 + vld, :], x2nb[:vld, :])
                nc.sync.dma_start(out_dram[r0:r0 + vld, D:2 * D],
                                  x2n[:vld, :])

            pend = None
            for b in range(B):
                cur = gat_block(b)
                if pend is not None:
                    gat_flush(b - 1, *pend)
                pend = cur
            gat_flush(B - 1, *pend)
            nc.leave_named_scope("gat", sc[0], False)

            sc = nc.enter_named_scope("ag2", False)
            nc.gpsimd.collective_compute(
                "AllGather", ALU.bypass, replica_groups=rg,
                ins=[cc2_in[:]], outs=[cc2_out[:]])
            nc.leave_named_scope("ag2", sc[0], False)

            # =================== Stage 3: RGCN ===================
            # per (tile, relation): Sr = onehot(dst) * mask_r/cnt_r (bf16 4x),
            # aggT_r[c, n] += xg^T Sr; per block: px3 = wroot^T x2T +
            # sum_r Wr^T aggT_r accumulated in one PSUM chain.
            sc = nc.enter_named_scope("rgcn", False)
            rgcn_tile = make_stage(
                "r", "xgr", rgcn, (cc2_out[0:HSPLIT, :], cc2_out[HSPLIT:N, :]),
                D, G_E, bufs=3)
            RTAGS = ["acc", "acc2", "d", "acc", "acc2"]
            RBUFS = {"acc": 2, "acc2": 2, "d": 1}

            def rgcn_block(b):
                runs = rgcn_runs[b]
                present = sorted({r for (_, _, r) in runs})
                pr = {}
                first = {r: True for r in present}
                lastrun = {}
                for i, (h, t, r) in enumerate(runs):
                    lastrun[r] = i
                for i, (h, t, r) in enumerate(runs):
                    if r not in pr:
                        tg = RTAGS[present.index(r)]
                        pr[r] = ps.tile([P, P], f32, name=f"pr{r}",
                                        tag=tg, bufs=RBUFS[tg])
                    Sr = sb.tile([P, P], bf16, tag="S", bufs=6)
                    nc.vector.tensor_scalar(
                        out=Sr[:], in0=iota_b[:],
                        scalar1=meta_sb[f"rd{h}"][:, t:t + 1],
                        scalar2=meta_sb[f"rw{h}"][:, t * R + r:t * R + r + 1],
                        op0=ALU.is_equal, op1=ALU.mult)
                    xg = rgcn_tile(h, t)
                    nc.tensor.matmul(pr[r][:], lhsT=xg, rhs=Sr[:],
                                     start=first[r], stop=(lastrun[r] == i))
                    first[r] = False
                return present, pr

            def rgcn_epilogue(b, present, pr):
                vld = min(P, SH - b * P)
                r0 = b * P
                px3 = ps.tile([P, P], f32, tag="mm", bufs=2)
                nc.tensor.matmul(px3[:], lhsT=wroot_sb[:],
                                 rhs=x2Tb_sb[:, r0:r0 + P],
                                 start=True, stop=(not present))
                for i, r in enumerate(present):
                    scr = sb.tile([P, P], bf16, tag="scr", bufs=3)
                    nc.scalar.copy(scr[:], pr[r][:])
                    nc.tensor.matmul(px3[:],
                                     lhsT=wr_sb[:, r * D:(r + 1) * D],
                                     rhs=scr[:], start=False,
                                     stop=(i == len(present) - 1))
                x3T = sb.tile([P, P], f32, tag="x3T")
                nc.scalar.activation(x3T[:], px3[:], AF.Identity,
                                     bias=br_sb[:, 0:1])
                ptr3 = ps.tile([P, P], f32, tag="mm", bufs=2)
                nc.tensor.transpose(ptr3[:], x3T[:], ident[:])
                x3n = sb.tile([P, P], f32, tag="x3n")
                nc.scalar.copy(x3n[:], ptr3[:])
                nc.sync.dma_start(out_dram[r0:r0 + vld, 2 * D:3 * D],
                                  x3n[:vld, :])

            for b in range(B):
                present, pr = rgcn_block(b)
                rgcn_epilogue(b, present, pr)
            nc.leave_named_scope("rgcn", sc[0], False)

    nc.compile()
    return nc


def kernel(x, edge_index, edge_type, W_sage_l, b_sage, W_sage_r,
           W_gat, att_src, att_dst, b_gat, W_rgcn, W_root, b_rgcn,
           _trace=False, _tmpdir=None):
    x = np.asarray(x, np.float32)
    edge_index = np.asarray(edge_index)
    edge_type = np.asarray(edge_type)

    pp = _preprocess(x, edge_index, edge_type)
    nc = _build_program(pp)

    W_gat = np.asarray(W_gat, np.float32)
    v = np.empty((D, 4), np.float32)
    for h in range(H):
        v[:, h] = W_gat[:, h, :] @ np.asarray(att_src, np.float32)[h]
        v[:, 2 + h] = W_gat[:, h, :] @ np.asarray(att_dst, np.float32)[h]

    common = {
        "xb": x.astype(BF),
        "wsl": np.asarray(W_sage_l, np.float32).astype(BF),
        "wsr": np.asarray(W_sage_r, np.float32).astype(BF),
        "bs": np.asarray(b_sage, np.float32).reshape(P, 1),
        "vsd": v,
        "wg0": W_gat[:, 0, :].astype(BF),
        "wg1": W_gat[:, 1, :].astype(BF),
        "bg": np.asarray(b_gat, np.float32).reshape(P, 1),
        "wroot": np.asarray(W_root, np.float32).astype(BF),
        "wr": np.asarray(W_rgcn, np.float32).astype(BF),
        "br": np.asarray(b_rgcn, np.float32).reshape(P, 1),
    }

    in_maps = []
    for k in range(NCORES):
        xs = np.zeros((B * P, D), np.float32)
        xs[:SH] = x[k * SH:(k + 1) * SH]
        m = dict(common)
        m["xt"] = np.ascontiguousarray(
            xs.reshape(B, P, D).transpose(0, 2, 1)).astype(BF)
        for h in range(2):
            if pp["edges"]["T"][h]:
                idx16, dlp, _, ewp, stp = pp["edges"]["cores"][k][h]
                m[f"ei{h}"] = idx16
                m[f"ed{h}"] = dlp
                m[f"ew{h}"] = ewp
                m[f"st{h}"] = stp
            if pp["rgcn"]["T"][h]:
                idx16, dlp, rwp, _, _ = pp["rgcn"]["cores"][k][h]
                m[f"ri{h}"] = idx16
                m[f"rd{h}"] = dlp
                m[f"rw{h}"] = rwp
        in_maps.append(m)

    res = run_bass_kernel_spmd(nc, in_maps, core_ids=list(range(NCORES)),
                               trace=_trace, tmpdir=_tmpdir)
    out = np.concatenate([res.results[k]["out"] for k in range(NCORES)], 0)
    if _trace:
        return out, res
    return out



# revision 13
# speedup vs baseline: 1.0976x; 1.0976x over previous
"""Trainium2 Bass kernel for ClassForgeEnsembleGNN (SAGE -> GAT -> RGCN ensemble).

Strategy (8 NeuronCores, SPMD):
  - Nodes partitioned into 8 contiguous shards (6250 each); each core owns the
    edges whose target is in its shard.  Weights replicated; x1/x2 node
    features all-gathered between stages (device collectives).
  - Per-edge source rows fetched with batched GPSIMD dma_gather (int16
    indices; tables addressed through two views split at 32768; each
    (block, half) edge group padded to a uniform tile count across cores).
  - Scatter-aggregation via selection-matrix matmuls.  All selection matrices
    (one-hot with host-known weights folded in) are precomputed on the host
    and streamed from DRAM as bf16 tiles — the DVE never builds them.
  - GAT: per-edge a_d via matmuls against host-precomputed transposed
    one-hot tiles; leaky-relu/exp batched per gather group; exp(logit)
    applied by scaling the gathered rows (head0 on DVE, head1 on ScalarE);
    both heads aggregated in a single [128, 258] matmul per tile against the
    streamed plain one-hot; self-loops as diag(exp) tiles built by one bf16
    multiply against a resident identity.
  - RGCN accumulates root + per-relation terms in PSUM; per-relation
    aggregation matmuls contract over partition sub-ranges of a shared
    streamed S tile (relation runs are slot-aligned within each tile).
"""

import sys
import os

for _p in ("/opt/trn_rl_repo", "/root/.axon_site/_ro/trn_rl_repo"):
    if os.path.isdir(_p) and _p not in sys.path:
        sys.path.append(_p)

import numpy as np
import ml_dtypes

import concourse.bacc as bacc
import concourse.bass as bass
import concourse.mybir as mybir
import concourse.tile as tile
from concourse.bass_utils import run_bass_kernel_spmd
from concourse.masks import make_identity

P = 128
NCORES = 8
N = 50000
E = 400000
D = 128
H = 2
R = 5
NEG = 0.2
SH = N // NCORES            # 6250 nodes per shard
B = (SH + P - 1) // P       # 49 blocks (last has 106 valid nodes)
HSPLIT = 32768              # int16-safe table split
G_E = 8                     # tiles per dma_gather group

f32 = mybir.dt.float32
bf16 = mybir.dt.bfloat16
i32 = mybir.dt.int32
i16 = mybir.dt.int16
AF = mybir.ActivationFunctionType
ALU = mybir.AluOpType
BF = ml_dtypes.bfloat16


def _pack_stage(src_pc, dst_pc, rel_pc, nrel, wslot_fn):
    """Lay out each core's edges into uniform tile slots.

    Grouping: (half = src >= HSPLIT, block = dst_local // 128 [, relation]).
    Sections (half, block) are tile-aligned; relation runs inside a section
    are slot-aligned (uniform across cores).

    Returns per-core packed arrays: int16 gather indices, streamed S tiles
    (weighted one-hot, [P, T*P] bf16) and, for nrel == 1, the transposed
    one-hot ST tiles.
    """
    counts = np.zeros((NCORES, 2, B, nrel), np.int64)
    for k in range(NCORES):
        h = (src_pc[k] >= HSPLIT).astype(np.int64)
        blk = dst_pc[k] // P
        r = rel_pc[k] if nrel > 1 else np.zeros(len(src_pc[k]), np.int64)
        np.add.at(counts[k], (h, blk, r), 1)
    caps = counts.max(0)                       # [2, B, nrel] slot caps
    run_off = np.zeros((2, B, nrel + 1), np.int64)
    np.cumsum(caps, axis=2, out=run_off[:, :, 1:])
    sec_slots = run_off[:, :, nrel]            # [2, B]
    sec_tiles = (sec_slots + P - 1) // P       # [2, B]
    tile_off = np.zeros((2, B + 1), np.int64)  # tile offset within stream
    np.cumsum(sec_tiles, axis=1, out=tile_off[:, 1:])
    T = tile_off[:, B]                         # [2] tiles per stream

    cores = []
    for k in range(NCORES):
        h = (src_pc[k] >= HSPLIT).astype(np.int64)
        blk = dst_pc[k] // P
        r = rel_pc[k] if nrel > 1 else np.zeros(len(src_pc[k]), np.int64)
        gid = (h * B + blk) * nrel + r
        order = np.argsort(gid, kind="stable")
        gs = gid[order]
        cnt_flat = counts[k].reshape(-1)
        starts = np.concatenate([[0], np.cumsum(cnt_flat)])[:-1]
        rank = np.arange(len(gs)) - starts[gs]
        hh, rest = gs // (B * nrel), gs % (B * nrel)
        bb, rr = rest // nrel, rest % nrel
        slot = tile_off[hh, bb] * P + run_off[hh, bb, rr] + rank
        w_all = wslot_fn(k, dst_pc[k], rel_pc[k] if nrel > 1 else None)
        packed = []
        for hv in range(2):
            n_slots = int(T[hv]) * P
            idx = np.zeros(n_slots, np.int64)
            dl = np.full(n_slots, -1.0, np.float32)
            wv = np.zeros(n_slots, np.float32)
            sel = hh == hv
            s = slot[sel]  # slot is relative to its stream's start
            idx[s] = src_pc[k][order][sel] - (HSPLIT if hv else 0)
            dl[s] = (dst_pc[k][order][sel] % P).astype(np.float32)
            wv[s] = w_all[order][sel]
            # idx packed for dma_gather: flat i -> (channel i%16, col i//16),
            # replicated down the 8 Q7 core groups
            m = idx.reshape(-1, 16).T.astype(np.int16)
            idx16 = np.tile(m, (8, 1))
            nt = int(T[hv])
            # streamed weighted one-hot S[e, n] = w_e * (dst_e == n);
            # layout [P(slot within tile), nt*P]
            dlt = dl.reshape(nt, P)
            wvt = wv.reshape(nt, P)
            if nt:
                oh = (dlt[:, :, None] ==
                      np.arange(P, dtype=np.float32)[None, None, :])
                sw = oh * wvt[:, :, None]                 # [nt, Pe, Pn]
                swp = np.ascontiguousarray(
                    sw.transpose(1, 0, 2).reshape(P, nt * P)).astype(BF)
                if nrel == 1:
                    st = np.ascontiguousarray(
                        sw.transpose(2, 0, 1).reshape(P, nt * P)).astype(BF)
                else:
                    st = None
            else:
                swp = np.zeros((P, 0), BF)
                st = np.zeros((P, 0), BF) if nrel == 1 else None
            packed.append((np.ascontiguousarray(idx16), swp, st,
                           dlt, wvt))
        cores.append(packed)

    return dict(caps=caps, run_off=run_off, sec_tiles=sec_tiles,
                tile_off=tile_off, T=(int(T[0]), int(T[1])), cores=cores)


def _preprocess(x, edge_index, edge_type):
    src = edge_index[0].astype(np.int64)
    dst = edge_index[1].astype(np.int64)
    et = edge_type.astype(np.int64)

    cnt = np.bincount(dst, minlength=N).astype(np.float32)
    wrec_all = 1.0 / np.maximum(cnt, 1.0)
    cnt_r = np.zeros((R, N), np.float32)
    for r in range(R):
        cnt_r[r] = np.bincount(dst[et == r], minlength=N)
    wrrec_all = 1.0 / np.maximum(cnt_r, 1.0)

    shard_of = dst // SH
    s_src, s_dst, r_rel = [], [], []
    for k in range(NCORES):
        sel = shard_of == k
        s_src.append(src[sel])
        s_dst.append(dst[sel] - k * SH)
        r_rel.append(et[sel])

    def w_sage(k, dst_local, rel):
        return wrec_all[k * SH + dst_local]

    def w_one(k, dst_local, rel):
        return np.ones(len(dst_local), np.float32)

    def w_rgcn(k, dst_local, rel):
        return wrrec_all[rel, k * SH + dst_local]

    edges = _pack_stage(s_src, s_dst, [None] * NCORES, 1, w_sage)
    gat = _pack_stage(s_src, s_dst, [None] * NCORES, 1, w_one)
    rgcn = _pack_stage(s_src, s_dst, r_rel, R, w_rgcn)
    jobs, jobs_per_block = _rgcn_jobs(rgcn)
    J = len(jobs)
    arange = np.arange(P, dtype=np.float32)
    srj_pc = []
    for k in range(NCORES):
        srj = np.zeros((J, P, P), np.float32)
        for q, (h, t, r, lo, hi) in enumerate(jobs):
            _, _, _, dlt, wvt = rgcn["cores"][k][h]
            dlr = dlt[t, lo:hi]
            wvr = wvt[t, lo:hi]
            srj[q, lo:hi, :] = (dlr[:, None] == arange[None, :]) \
                * wvr[:, None]
        srj_pc.append(np.ascontiguousarray(
            srj.transpose(1, 0, 2).reshape(P, J * P)).astype(BF))
    rgcn["jobs"] = jobs
    rgcn["jobs_per_block"] = jobs_per_block
    rgcn["J"] = J
    rgcn["srj_pc"] = srj_pc
    return dict(edges=edges, gat=gat, rgcn=rgcn)


def _rgcn_jobs(rgcn):
    """Global job list ordered by block: (h, tile, r, lo, hi)."""
    jobs = []
    jobs_per_block = []
    for b in range(B):
        blk = []
        for h in range(2):
            t0 = int(rgcn["tile_off"][h, b])
            ntile = int(rgcn["sec_tiles"][h, b])
            for tl in range(ntile):
                s0, s1 = tl * P, (tl + 1) * P
                for r in range(R):
                    lo = int(rgcn["run_off"][h, b, r])
                    hi = int(rgcn["run_off"][h, b, r + 1])
                    if lo < s1 and hi > s0:
                        blk.append((h, t0 + tl, r,
                                    max(lo, s0) - s0, min(hi, s1) - s0))
        jobs_per_block.append(blk)
        jobs.extend(blk)
    return jobs, jobs_per_block


def _rgcn_runs(rgcn):
    """Per block: list of (h, stream_tile, r, lo, hi) matmul jobs where
    [lo, hi) is the slot range of relation r within the tile."""
    runs_per_block = []
    for b in range(B):
        runs = []
        for h in range(2):
            t0 = int(rgcn["tile_off"][h, b])
            ntile = int(rgcn["sec_tiles"][h, b])
            for tl in range(ntile):
                s0, s1 = tl * P, (tl + 1) * P
                for r in range(R):
                    lo = int(rgcn["run_off"][h, b, r])
                    hi = int(rgcn["run_off"][h, b, r + 1])
                    if lo < s1 and hi > s0:
                        runs.append((h, t0 + tl, r,
                                     max(lo, s0) - s0, min(hi, s1) - s0))
        runs_per_block.append(runs)
    return runs_per_block


def _build_program(pp):
    edges, gat, rgcn = pp["edges"], pp["gat"], pp["rgcn"]

    nc = bacc.Bacc("TRN2", target_bir_lowering=False, debug=False,
                   num_devices=NCORES, num_swdge_queues=4)

    xb_dram = nc.dram_tensor("xb", [N, D], bf16, kind="ExternalInput")
    xt_dram = nc.dram_tensor("xt", [B, P, P], bf16, kind="ExternalInput")
    meta = {}
    for h in range(2):
        T = edges["T"][h]
        if T:
            meta[f"ei{h}"] = nc.dram_tensor(
                f"ei{h}", [P, T * 8], i16, kind="ExternalInput")
            meta[f"se{h}"] = nc.dram_tensor(
                f"se{h}", [P, T * P], bf16, kind="ExternalInput")
            meta[f"sg{h}"] = nc.dram_tensor(
                f"sg{h}", [P, T * P], bf16, kind="ExternalInput")
            meta[f"st{h}"] = nc.dram_tensor(
                f"st{h}", [P, T * P], bf16, kind="ExternalInput")
        T = rgcn["T"][h]
        if T:
            meta[f"ri{h}"] = nc.dram_tensor(
                f"ri{h}", [P, T * 8], i16, kind="ExternalInput")
    meta["srj"] = nc.dram_tensor(
        "srj", [P, rgcn["J"] * P], bf16, kind="ExternalInput")
    wsl = nc.dram_tensor("wsl", [D, D], bf16, kind="ExternalInput")
    wsr = nc.dram_tensor("wsr", [D, D], bf16, kind="ExternalInput")
    bs = nc.dram_tensor("bs", [P, 1], f32, kind="ExternalInput")
    vsd = nc.dram_tensor("vsd", [D, 4], f32, kind="ExternalInput")
    wg0 = nc.dram_tensor("wg0", [D, D], bf16, kind="ExternalInput")
    wg1 = nc.dram_tensor("wg1", [D, D], bf16, kind="ExternalInput")
    bg = nc.dram_tensor("bg", [P, 1], f32, kind="ExternalInput")
    wroot = nc.dram_tensor("wroot", [D, D], bf16, kind="ExternalInput")
    wr_d = nc.dram_tensor("wr", [R, D, D], bf16, kind="ExternalInput")
    br = nc.dram_tensor("br", [P, 1], f32, kind="ExternalInput")
    out_dram = nc.dram_tensor("out", [SH, 3 * D], f32, kind="ExternalOutput")

    rg = [list(range(NCORES))]
    qrr = [0]

    def next_q():
        q = qrr[0]
        qrr[0] = (q + 1) % 4
        return q

    XSTRIDE = 132   # x1 row block stride: [x1(128) | 1 | as0 | as1 | pad]

    with tile.TileContext(nc) as tc:
        with (
            tc.tile_pool(name="const", bufs=1) as cb,
            tc.tile_pool(name="sbuf", bufs=3) as sb,
            tc.tile_pool(name="psum", bufs=1, space="PSUM") as ps,
            tc.tile_pool(name="dram", bufs=1, space="DRAM") as dr,
        ):
            identb = cb.tile([P, P], bf16)
            make_identity(nc, identb[:])
            ident = cb.tile([P, P], f32)
            make_identity(nc, ident[:])

            def load_const(name, dram, dtype):
                t = cb.tile(list(dram.shape), dtype, name=name)
                nc.sync.dma_start(t[:], dram[:])
                return t

            wsl_sb = load_const("wsl_sb", wsl, bf16)
            wsr_sb = load_const("wsr_sb", wsr, bf16)
            bs_sb = load_const("bs_sb", bs, f32)
            vsd_sb = load_const("vsd_sb", vsd, f32)
            wg0_sb = load_const("wg0_sb", wg0, bf16)
            wg1_sb = load_const("wg1_sb", wg1, bf16)
            bg_sb = load_const("bg_sb", bg, f32)
            wroot_sb = load_const("wroot_sb", wroot, bf16)
            br_sb = load_const("br_sb", br, f32)
            wr_sb = cb.tile([P, R * D], bf16)
            for r in range(R):
                nc.sync.dma_start(wr_sb[:, r * D:(r + 1) * D], wr_d[r, :, :])
            idx_sb = {}
            for name in ("ei0", "ei1", "ri0", "ri1"):
                if name in meta:
                    dram = meta[name]
                    t = cb.tile(list(dram.shape), dram.dtype, name=f"{name}_sb")
                    nc.sync.dma_start(t[:], dram[:])
                    idx_sb[name] = t

            # persistent per-shard state
            x1nb_sb = cb.tile([P, B * XSTRIDE], bf16)  # [x1 | 1 | as] / block
            adb_sb = cb.tile([P, 2 * B], bf16)   # a_d bf16 / block
            exl_sb = cb.tile([P, 2 * B], f32)   # exp(leaky(as+ad)) / block
            x2Tb_sb = cb.tile([P, B * P], bf16)  # x2 transposed bf16

            cc1_in = dr.tile([SH, 256], bf16)
            cc1_out = dr.tile([N, 256], bf16, addr_space="Shared")
            zpad = cb.tile([P, 256 - 131], bf16)
            nc.vector.memset(zpad[:], 0.0)
            for b in range(B):
                vld = min(P, SH - b * P)
                nc.sync.dma_start(cc1_in[b * P:b * P + vld, 131:256],
                                  zpad[:vld, :])
            cc2_in = dr.tile([SH, D], bf16)
            cc2_out = dr.tile([N, D], bf16, addr_space="Shared")

            # ---------- gather + stream machinery ----------
            def make_stage(key, tag, d_, table_views, width, bufs,
                           streams=()):
                """Group-batched gathers plus aligned S-tile streams.

                streams: list of (meta key prefix, tag) to stream
                [P, gtiles*P] bf16 alongside each gather group.
                Returns (get_tile, get_stream(prefix)) accessors.
                """
                state = {}

                def ensure(h, g):
                    k2 = (h, g)
                    if k2 not in state:
                        T = d_["T"][h]
                        g0 = g * G_E
                        gn = min(G_E, T - g0)
                        xg = sb.tile([P, G_E, width], bf16,
                                     tag=tag, bufs=bufs)
                        ni = gn * P
                        nc.gpsimd.dma_gather(
                            xg[:, 0:gn, :], table_views[h],
                            idx_sb[f"{key}i{h}"][:, g0 * 8:(g0 + gn) * 8],
                            ni, ni, width, queue_num=next_q())
                        ss = {}
                        for pre, stag in streams:
                            s = sb.tile([P, G_E, P], bf16, tag=stag,
                                        bufs=bufs)
                            nc.sync.dma_start(
                                s[:, 0:gn, :],
                                meta[f"{pre}{h}"][:, g0 * P:(g0 + gn) * P])
                            ss[pre] = s
                        state[k2] = (xg, ss)
                    return state[k2]

                def get_tile(h, t):
                    g = t // G_E
                    xg, _ = ensure(h, g)
                    return xg[:, t - g * G_E, :]

                def get_stream(pre, h, t):
                    g = t // G_E
                    _, ss = ensure(h, g)
                    return ss[pre][:, t - g * G_E, :]

                def get_group(h, g):
                    return ensure(h, g)

                return get_tile, get_stream, get_group

            def block_tiles(d_, b):
                return [(h, t) for h in range(2)
                        for t in range(int(d_["tile_off"][h, b]),
                                       int(d_["tile_off"][h, b + 1]))]

            # =================== Stage 1: SAGE ===================
            # per tile: aggT[c, n] += xg^T S (S streamed, weights folded).
            sc = nc.enter_named_scope("sage", False)
            sage_tile, sage_stream, _ = make_stage(
                "e", "xgs", edges, (xb_dram[0:HSPLIT, :], xb_dram[HSPLIT:N, :]),
                D, 3, streams=(("se", "Se"),))

            def sage_block(b):
                tiles = block_tiles(edges, b)
                pa = ps.tile([P, P], f32, tag="acc", bufs=2)
                for j, (h, t) in enumerate(tiles):
                    xg = sage_tile(h, t)
                    S = sage_stream("se", h, t)
                    nc.tensor.matmul(pa[:], lhsT=xg, rhs=S,
                                     start=(j == 0),
                                     stop=(j == len(tiles) - 1))
                if not tiles:
                    nc.vector.memset(pa[:], 0.0)
                return pa

            def sage_epilogue(b, pa):
                vld = min(P, SH - b * P)
                r0 = b * P
                x0 = b * XSTRIDE
                aggT = sb.tile([P, P], bf16, tag="aggT")
                nc.scalar.copy(aggT[:], pa[:])
                xT = sb.tile([P, P], bf16, tag="xT")
                nc.sync.dma_start(xT[:], xt_dram[b, :, :])
                pm = ps.tile([P, P], f32, tag="mm", bufs=2)
                nc.tensor.matmul(pm[:], lhsT=wsl_sb[:], rhs=aggT[:],
                                 start=True, stop=False)
                nc.tensor.matmul(pm[:], lhsT=wsr_sb[:], rhs=xT[:],
                                 start=False, stop=True)
                x1T = sb.tile([P, P], f32, tag="x1T")
                nc.scalar.activation(x1T[:], pm[:], AF.Relu,
                                     bias=bs_sb[:, 0:1], scale=1.0)
                pasd = ps.tile([P, 4], f32, tag="c", bufs=1)
                nc.tensor.matmul(pasd[:], lhsT=x1T[:], rhs=vsd_sb[:],
                                 start=True, stop=True)
                asd4 = sb.tile([P, 4], f32, tag="asd4")
                nc.scalar.copy(asd4[:], pasd[:])
                nc.scalar.copy(adb_sb[:, 2 * b:2 * b + 2], pasd[:, 2:4])
                evl = sb.tile([P, 2], f32, tag="evl")
                nc.vector.tensor_add(evl[:], asd4[:, 0:2], asd4[:, 2:4])
                lvl = sb.tile([P, 2], f32, tag="lvl")
                nc.vector.scalar_tensor_tensor(
                    out=lvl[:], in0=evl[:], scalar=NEG, in1=evl[:],
                    op0=ALU.mult, op1=ALU.max)
                nc.scalar.activation(exl_sb[:, 2 * b:2 * b + 2], lvl[:],
                                     AF.Exp)
                ptr2 = ps.tile([P, P], f32, tag="mm", bufs=2)
                nc.tensor.transpose(ptr2[:], x1T[:], ident[:])
                nc.scalar.copy(x1nb_sb[:, x0:x0 + P], ptr2[:])
                nc.vector.memset(x1nb_sb[:, x0 + P:x0 + P + 1], 1.0)
                nc.scalar.copy(x1nb_sb[:, x0 + 129:x0 + 131], asd4[:, 0:2])
                x1n = sb.tile([P, P], f32, tag="x1n")
                nc.scalar.copy(x1n[:], ptr2[:])
                nc.sync.dma_start(cc1_in[r0:r0 + vld, 0:131],
                                  x1nb_sb[:vld, x0:x0 + 131])
                nc.sync.dma_start(out_dram[r0:r0 + vld, 0:D], x1n[:vld, :])

            pend = None
            for b in range(B):
                pa = sage_block(b)
                if pend is not None:
                    sage_epilogue(b - 1, pend)
                pend = pa
            sage_epilogue(B - 1, pend)
            nc.leave_named_scope("sage", sc[0], False)

            sc = nc.enter_named_scope("ag1", False)
            nc.gpsimd.collective_compute(
                "AllGather", ALU.bypass, replica_groups=rg,
                ins=[cc1_in[:]], outs=[cc1_out[:]])
            nc.leave_named_scope("ag1", sc[0], False)

            # =================== Stage 2: GAT ===================
            sc = nc.enter_named_scope("gat", False)
            gat_tile, gat_stream, gat_group = make_stage(
                "e", "xgg", gat, (cc1_out[0:HSPLIT, :], cc1_out[HSPLIT:N, :]),
                256, 4, streams=(("sg", "Sg"), ("st", "STg")))

            # per gather group: a_d select (ST matmuls into one PSUM),
            # batched ev/leaky/exp; returns exf tiles (f32 + bf16).
            grp_state = {}

            def gat_prep_group(h, g):
                k2 = (h, g)
                if k2 in grp_state:
                    return grp_state[k2]
                T = gat["T"][h]
                g0 = g * G_E
                gn = min(G_E, T - g0)
                xg, ss = gat_group(h, g)
                stt = ss["st"]
                padp = ps.tile([P, 2 * G_E], f32, tag="acc2", bufs=2)
                for j in range(gn):
                    t = g0 + j
                    # which block does tile t belong to?
                    b = int(np.searchsorted(gat["tile_off"][h], t,
                                            side="right")) - 1
                    nc.tensor.matmul(padp[:, 2 * j:2 * j + 2],
                                     lhsT=stt[:, j, :],
                                     rhs=adb_sb[:, 2 * b:2 * b + 2],
                                     start=True, stop=True,
                                     skip_group_check=True)
                pads = sb.tile([P, G_E, 2], f32, tag="pads", bufs=4)
                nc.scalar.copy(pads[:, 0:gn, :], padp[:, 0:2 * gn])
                ev = sb.tile([P, G_E, 2], f32, tag="ev", bufs=4)
                nc.vector.tensor_add(ev[:, 0:gn, :],
                                     xg[:, 0:gn, 129:131],
                                     pads[:, 0:gn, :])
                lv = sb.tile([P, G_E, 2], f32, tag="lv", bufs=4)
                nc.vector.scalar_tensor_tensor(
                    out=lv[:, 0:gn, :], in0=ev[:, 0:gn, :], scalar=NEG,
                    in1=ev[:, 0:gn, :], op0=ALU.mult, op1=ALU.max)
                exf = sb.tile([P, G_E, 2], f32, tag="exf", bufs=4)
                nc.scalar.activation(exf[:, 0:gn, :], lv[:, 0:gn, :], AF.Exp)
                grp_state[k2] = exf
                return exf

            def gat_block(b):
                """Scaled tiles [xg*exf0 | xg*exf1] plus streamed one-hot for
                each tile of block b; self-loop diag tiles."""
                tiles = block_tiles(gat, b)
                jobs = []
                for (h, t) in tiles:
                    g = t // G_E
                    exf = gat_prep_group(h, g)
                    j = t - g * G_E
                    xg = gat_tile(h, t)
                    sgt = gat_stream("sg", h, t)
                    x01 = sb.tile([P, 258], bf16, tag="x01", bufs=28)
                    nc.vector.tensor_scalar(
                        out=x01[:, 0:129], in0=xg[:, 0:129],
                        scalar1=exf[:, j, 0:1], scalar2=None,
                        op0=ALU.mult)
                    nc.scalar.activation(x01[:, 129:258], xg[:, 0:129],
                                         AF.Identity, scale=exf[:, j, 1:2])
                    jobs.append((sgt, x01))
                # self-loop tiles: S = diag(exl_h)
                Ss = []
                for hh in range(2):
                    Sd = sb.tile([P, P], bf16, tag="Sd", bufs=4)
                    nc.vector.tensor_scalar(
                        out=Sd[:], in0=identb[:],
                        scalar1=exl_sb[:, 2 * b + hh:2 * b + hh + 1],
                        scalar2=None, op0=ALU.mult)
                    Ss.append(Sd)
                return jobs, Ss

            def gat_flush(b, jobs, Ss):
                vld = min(P, SH - b * P)
                r0 = b * P
                x0 = b * XSTRIDE
                p01 = ps.tile([P, 258], f32, tag="acc", bufs=2)
                n = len(jobs)
                for j, (sgt, x01) in enumerate(jobs):
                    nc.tensor.matmul(p01[:], lhsT=sgt, rhs=x01[:],
                                     start=(j == 0), stop=False,
                                     skip_group_check=True)
                nc.tensor.matmul(p01[:, 0:129], lhsT=Ss[0][:],
                                 rhs=x1nb_sb[:, x0:x0 + 129],
                                 start=(n == 0), stop=True,
                                 skip_group_check=True)
                nc.tensor.matmul(p01[:, 129:258], lhsT=Ss[1][:],
                                 rhs=x1nb_sb[:, x0:x0 + 129],
                                 start=(n == 0), stop=True,
                                 skip_group_check=True)
                den = sb.tile([P, 2], f32, tag="den")
                nc.scalar.copy(den[:, 0:1], p01[:, 128:129])
                nc.scalar.copy(den[:, 1:2], p01[:, 257:258])
                rec = sb.tile([P, 2], f32, tag="rec")
                nc.vector.reciprocal(rec[:], den[:])
                tb = []
                for hh in range(2):
                    nmh = sb.tile([P, P], bf16, tag=f"nm{hh}")
                    nc.scalar.activation(nmh[:], p01[:, 129 * hh:129 * hh + 128],
                                         AF.Identity, scale=rec[:, hh:hh + 1])
                    ptn = ps.tile([P, P], bf16, tag="mm", bufs=2)
                    nc.tensor.transpose(ptn[:], nmh[:], identb[:])
                    tbh = sb.tile([P, P], bf16, tag=f"tb{hh}")
                    nc.scalar.copy(tbh[:], ptn[:])
                    tb.append(tbh)
                px2 = ps.tile([P, P], f32, tag="mm", bufs=2)
                nc.tensor.matmul(px2[:], lhsT=wg0_sb[:], rhs=tb[0][:],
                                 start=True, stop=False)
                nc.tensor.matmul(px2[:], lhsT=wg1_sb[:], rhs=tb[1][:],
                                 start=False, stop=True)
                nc.scalar.activation(x2Tb_sb[:, r0:r0 + P], px2[:], AF.Relu,
                                     bias=bg_sb[:, 0:1], scale=0.5)
                ptx = ps.tile([P, P], bf16, tag="mm", bufs=2)
                nc.tensor.transpose(ptx[:], x2Tb_sb[:, r0:r0 + P], identb[:])
                x2n = sb.tile([P, P], f32, tag="x2n")
                nc.scalar.copy(x2n[:], ptx[:])
                x2nb = sb.tile([P, P], bf16, tag="x2nb")
                nc.scalar.copy(x2nb[:], ptx[:])
                nc.sync.dma_start(cc2_in[r0:r0 + vld, :], x2nb[:vld, :])
                nc.sync.dma_start(out_dram[r0:r0 + vld, D:2 * D],
                                  x2n[:vld, :])

            pend = None
            for b in range(B):
                cur = gat_block(b)
                if pend is not None:
                    gat_flush(b - 1, *pend)
                pend = cur
            gat_flush(B - 1, *pend)
            nc.leave_named_scope("gat", sc[0], False)

            sc = nc.enter_named_scope("ag2", False)
            nc.gpsimd.collective_compute(
                "AllGather", ALU.bypass, replica_groups=rg,
                ins=[cc2_in[:]], outs=[cc2_out[:]])
            nc.leave_named_scope("ag2", sc[0], False)

            # =================== Stage 3: RGCN ===================
            # per (tile, relation run): aggT_r[c, n] += xg[lo:hi]^T S[lo:hi]
            # (S streamed, shared per tile, mask/cnt_r weights folded);
            # per block: px3 = wroot^T x2T + sum_r Wr^T aggT_r in one chain.
            sc = nc.enter_named_scope("rgcn", False)
            rgcn_tile, _, _ = make_stage(
                "r", "xgr", rgcn, (cc2_out[0:HSPLIT, :], cc2_out[HSPLIT:N, :]),
                D, 3)
            job_off = [0]
            for blk in rgcn["jobs_per_block"]:
                job_off.append(job_off[-1] + len(blk))
            RTAGS = ["acc", "acc2", "d", "acc", "acc2"]
            RBUFS = {"acc": 2, "acc2": 2, "d": 1}

            def rgcn_block(b):
                runs = rgcn["jobs_per_block"][b]
                nj = len(runs)
                q0 = job_off[b]
                srb = sb.tile([P, max(nj, 1), P], bf16, tag="Srb", bufs=3)
                if nj:
                    nc.sync.dma_start(
                        srb[:, 0:nj, :],
                        meta["srj"][:, q0 * P:(q0 + nj) * P])
                present = sorted({r for (_, _, r, _, _) in runs})
                pr = {}
                first = {r: True for r in present}
                lastrun = {}
                for i, (h, t, r, lo, hi) in enumerate(runs):
                    lastrun[r] = i
                for i, (h, t, r, lo, hi) in enumerate(runs):
                    if r not in pr:
                        tg = RTAGS[present.index(r)]
                        pr[r] = ps.tile([P, P], f32, name=f"pr{r}",
                                        tag=tg, bufs=RBUFS[tg])
                    xg = rgcn_tile(h, t)
                    nc.tensor.matmul(pr[r][:], lhsT=xg,
                                     rhs=srb[:, i, :],
                                     start=first[r], stop=(lastrun[r] == i))
                    first[r] = False
                return present, pr

            def rgcn_epilogue(b, present, pr):
                vld = min(P, SH - b * P)
                r0 = b * P
                px3 = ps.tile([P, P], f32, tag="mm", bufs=2)
                nc.tensor.matmul(px3[:], lhsT=wroot_sb[:],
                                 rhs=x2Tb_sb[:, r0:r0 + P],
                                 start=True, stop=(not present))
                for i, r in enumerate(present):
                    scr = sb.tile([P, P], bf16, tag="scr", bufs=3)
                    nc.scalar.copy(scr[:], pr[r][:])
                    nc.tensor.matmul(px3[:],
                                     lhsT=wr_sb[:, r * D:(r + 1) * D],
                                     rhs=scr[:], start=False,
                                     stop=(i == len(present) - 1))
                x3T = sb.tile([P, P], f32, tag="x3T")
                nc.scalar.activation(x3T[:], px3[:], AF.Identity,
                                     bias=br_sb[:, 0:1])
                ptr3 = ps.tile([P, P], f32, tag="mm", bufs=2)
                nc.tensor.transpose(ptr3[:], x3T[:], ident[:])
                x3n = sb.tile([P, P], f32, tag="x3n")
                nc.scalar.copy(x3n[:], ptr3[:])
                nc.sync.dma_start(out_dram[r0:r0 + vld, 2 * D:3 * D],
                                  x3n[:vld, :])

            for b in range(B):
                present, pr = rgcn_block(b)
                rgcn_epilogue(b, present, pr)
            nc.leave_named_scope("rgcn", sc[0], False)

    nc.compile()
    return nc


def kernel(x, edge_index, edge_type, W_sage_l, b_sage, W_sage_r,
           W_gat, att_src, att_dst, b_gat, W_rgcn, W_root, b_rgcn,
           _trace=False, _tmpdir=None):
    x = np.asarray(x, np.float32)
    edge_index = np.asarray(edge_index)
    edge_type = np.asarray(edge_type)

    pp = _preprocess(x, edge_index, edge_type)
    nc = _build_program(pp)

    W_gat = np.asarray(W_gat, np.float32)
    v = np.empty((D, 4), np.float32)
    for h in range(H):
        v[:, h] = W_gat[:, h, :] @ np.asarray(att_src, np.float32)[h]
        v[:, 2 + h] = W_gat[:, h, :] @ np.asarray(att_dst, np.float32)[h]

    common = {
        "xb": x.astype(BF),
        "wsl": np.asarray(W_sage_l, np.float32).astype(BF),
        "wsr": np.asarray(W_sage_r, np.float32).astype(BF),
        "bs": np.asarray(b_sage, np.float32).reshape(P, 1),
        "vsd": v,
        "wg0": W_gat[:, 0, :].astype(BF),
        "wg1": W_gat[:, 1, :].astype(BF),
        "bg": np.asarray(b_gat, np.float32).reshape(P, 1),
        "wroot": np.asarray(W_root, np.float32).astype(BF),
        "wr": np.asarray(W_rgcn, np.float32).astype(BF),
        "br": np.asarray(b_rgcn, np.float32).reshape(P, 1),
    }

    in_maps = []
    for k in range(NCORES):
        xs = np.zeros((B * P, D), np.float32)
        xs[:SH] = x[k * SH:(k + 1) * SH]
        m = dict(common)
        m["xt"] = np.ascontiguousarray(
            xs.reshape(B, P, D).transpose(0, 2, 1)).astype(BF)
        for h in range(2):
            if pp["edges"]["T"][h]:
                idx16, swp, _, _, _ = pp["edges"]["cores"][k][h]
                m[f"ei{h}"] = idx16
                m[f"se{h}"] = swp
                _, g_sw, g_st, _, _ = pp["gat"]["cores"][k][h]
                m[f"sg{h}"] = g_sw
                m[f"st{h}"] = g_st
            if pp["rgcn"]["T"][h]:
                idx16, _, _, _, _ = pp["rgcn"]["cores"][k][h]
                m[f"ri{h}"] = idx16
        m["srj"] = pp["rgcn"]["srj_pc"][k]
        in_maps.append(m)

    res = run_bass_kernel_spmd(nc, in_maps, core_ids=list(range(NCORES)),
                               trace=_trace, tmpdir=_tmpdir)
    out = np.concatenate([res.results[k]["out"] for k in range(NCORES)], 0)
    if _trace:
        return out, res
    return out
